# revision 1
# baseline (speedup 1.0000x reference)
"""Causal self-attention (B=2, T=2048, E=1024, H=16, D=64) on 8 NeuronCores.

Sharding: batch (2) x head-groups (4 groups of 4 heads) -> 8 cores.
Each core computes, for its (batch b, head group g):
  Q^T/K^T = (Wq/Wk col-shard)^T @ x_b^T        (heads on partitions, t free)
  V       = x_b @ Wv col-shard                 (tokens on partitions)
  S^T     = K^T-slices^T-matmuls (s on partitions, t free), causal
  P^T     = exp(S^T/8) (no max subtraction: scores ~ N(0,1), exp is safe)
  U^T,r   = [V | ones]^T @ P^T  (PV product + row-sums in one matmul)
  A^T     = U^T * (1/r)                        (softmax normalization)
  Ypart   = A^T-slices^T @ Wo row-shard        (partial out-proj)
Host sums the 4 partials per batch and adds the constant row
bo + bv @ Wo (exact: v-bias passes through attention unchanged; the k-bias
is softmax-invariant and dropped; q-bias is applied to Q on device).

v2: fully software-pipelined emission. Phase B (attention) is emitted so
the PE never head-of-line blocks on the ACT exp (S of block j+1 is queued
before PV of block j), and the projection matmuls of chunk c+1 plus the
out-proj matmuls of chunk c-1 are woven between attention blocks of chunk
c to fill PE slack while ACT computes exps. DMA is split across both
HWDGE queues (x loads on sync/SP, y stores on scalar/ACT) and batched into
large transfers. PSUM: s2 2x2 banks (double-buffered scores), u2 2 banks
(PV accumulator + row sums), ps_a 2x1 banks (projection/out-proj) = 8.

Matmuls run in float32r (single-pass reduced-precision fp32, ~2e-4 rel err).
"""
import sys

if "/opt/trn_rl_repo" not in sys.path:
    sys.path.insert(0, "/opt/trn_rl_repo")

import numpy as np

import concourse.bass as bass
import concourse.mybir as mybir
import concourse.tile as tile
from concourse import bacc
from concourse.bass_utils import run_bass_kernel_spmd

F32 = mybir.dt.float32
F32R = mybir.dt.float32r

B, T, E = 2, 2048, 1024
H, D = 16, 64
N_CORES = 8
HEADS_PER_CORE = 4            # 16 heads / 4 groups
EPC = HEADS_PER_CORE * D      # 256: e' columns per core
TC = 512                      # t-chunk (psum free width)
NTC = T // TC                 # 4 t-chunks
SB = 128                      # s-block (psum partitions)
NSB = T // SB                 # 16 s-blocks
KCH = E // 128                # 8 contraction chunks


def build_kernel(repeat: int = 1, debug_taps: bool = False) -> bass.Bass:
    """repeat>1 wraps the whole compute in a hardware loop — used only for
    wall-clock timing (per-iteration time is measured as
    (wall(R2) - wall(R1)) / (R2 - R1))."""
    nc = bacc.Bacc(None, target_bir_lowering=False, debug=False)

    xT = nc.dram_tensor("xT", [E, T], F32, kind="ExternalInput")
    wq = nc.dram_tensor("wq", [E, EPC], F32, kind="ExternalInput")
    wk = nc.dram_tensor("wk", [E, EPC], F32, kind="ExternalInput")
    wv = nc.dram_tensor("wv", [E, EPC], F32, kind="ExternalInput")
    wo = nc.dram_tensor("wo", [EPC, E], F32, kind="ExternalInput")
    bq = nc.dram_tensor("bq", [EPC], F32, kind="ExternalInput")
    onesc = nc.dram_tensor("onesc", [128, NSB * HEADS_PER_CORE], F32,
                           kind="ExternalInput")
    y = nc.dram_tensor("y", [T, E], F32, kind="ExternalOutput")
    taps = {}
    if debug_taps:
        for name in ("qt0", "qt1", "kt0", "kt1", "at0", "at1"):
            taps[name] = nc.dram_tensor(name, [128, T], F32,
                                        kind="ExternalOutput")
        taps["v"] = nc.dram_tensor(
            "v", [128, NSB, HEADS_PER_CORE, D + 1], F32,
            kind="ExternalOutput")

    xT4 = xT.rearrange("(k p) t -> k p t", p=128)  # [KCH, 128, T]

    with tile.TileContext(nc) as tc:
        with tc.tile_pool(name="singles", bufs=1) as singles, \
             tc.tile_pool(name="xtp", bufs=2) as xtp, \
             tc.tile_pool(name="pp", bufs=6) as pp, \
             tc.tile_pool(name="rip", bufs=4) as rip, \
             tc.tile_pool(name="rbp", bufs=2) as rbp, \
             tc.tile_pool(name="ysb", bufs=3) as ysbp, \
             tc.tile_pool(name="ps_a", bufs=2, space="PSUM") as ps_a, \
             tc.tile_pool(name="ps_s", bufs=2, space="PSUM") as ps_s, \
             tc.tile_pool(name="ps_u", bufs=1, space="PSUM") as ps_u:

            # ---- weight / bias loads (once, outside the timing loop) ----
            wq_sb = singles.tile([128, KCH, EPC], F32R, tag="wq")
            wk_sb = singles.tile([128, KCH, EPC], F32R, tag="wk")
            wv_sb = singles.tile([128, KCH, EPC], F32R, tag="wv")
            for k in range(KCH):
                nc.sync.dma_start(out=wq_sb[:, k, :],
                                  in_=wq[k * 128:(k + 1) * 128, :].bitcast(F32R))
                nc.sync.dma_start(out=wk_sb[:, k, :],
                                  in_=wk[k * 128:(k + 1) * 128, :].bitcast(F32R))
                nc.sync.dma_start(out=wv_sb[:, k, :],
                                  in_=wv[k * 128:(k + 1) * 128, :].bitcast(F32R))
            # wo: head h lives at partitions 64*(h%2).. of slab h//2
            wo_sb = singles.tile([128, 2, E], F32R, tag="wo")
            for j in range(2):
                nc.sync.dma_start(out=wo_sb[:, j, :],
                                  in_=wo[j * 128:(j + 1) * 128, :].bitcast(F32R))
            bq_sb = singles.tile([128, 2], F32, tag="bq")
            for eh in range(2):
                nc.sync.dma_start(out=bq_sb[:, eh],
                                  in_=bq[eh * 128:(eh + 1) * 128])

            # ---- persistent activations ----
            # QT/KT/AT: pair slab eh holds heads (2eh, 2eh+1) on partition
            # halves; V_all[s_part, s_block, head, 0:64]=V, [.,.,.,64]=1.0
            QT = [singles.tile([128, T], F32R, tag=f"QT{eh}", name=f"QT{eh}")
                  for eh in range(2)]
            KT = [singles.tile([128, T], F32R, tag=f"KT{eh}", name=f"KT{eh}")
                  for eh in range(2)]
            AT = [singles.tile([128, T], F32R, tag=f"AT{eh}", name=f"AT{eh}")
                  for eh in range(2)]
            V_all = singles.tile([128, NSB, HEADS_PER_CORE, D + 1], F32R,
                                 tag="V")
            # ones column (memset doesn't support f32r: DMA a constant in)
            nc.sync.dma_start(out=V_all[:, :, :, D:D + 1],
                              in_=onesc[:].bitcast(F32R))

            def emit_body():
                xts = [None] * NTC

                def dma_x(c):
                    xt = xtp.tile([128, KCH, TC], F32R, tag="xt")
                    t0 = c * TC
                    nc.sync.dma_start(
                        out=xt[:],
                        in_=xT4[:, :, t0:t0 + TC].rearrange(
                            "k p t -> p k t").bitcast(F32R))
                    xts[c] = xt

                # ---- phase A unit: one projection group for chunk c ----
                def a_unit(c, kind, idx):
                    t0 = c * TC
                    xt = xts[c]
                    if kind == "q":
                        q_ps = ps_a.tile([128, TC], F32, tag="a")
                        for k in range(KCH):
                            nc.tensor.matmul(
                                q_ps[:], wq_sb[:, k, idx * 128:(idx + 1) * 128],
                                xt[:, k, :], start=(k == 0),
                                stop=(k == KCH - 1))
                        nc.vector.tensor_scalar_add(
                            out=QT[idx][:, t0:t0 + TC], in0=q_ps[:],
                            scalar1=bq_sb[:, idx:idx + 1])
                    elif kind == "k":
                        k_ps = ps_a.tile([128, TC], F32, tag="a")
                        for k in range(KCH):
                            nc.tensor.matmul(
                                k_ps[:], wk_sb[:, k, idx * 128:(idx + 1) * 128],
                                xt[:, k, :], start=(k == 0),
                                stop=(k == KCH - 1))
                        nc.vector.tensor_copy(KT[idx][:, t0:t0 + TC], k_ps[:])
                    else:  # "v"
                        v_ps = ps_a.tile([128, EPC], F32, tag="a")
                        for k in range(KCH):
                            nc.tensor.matmul(
                                v_ps[:], xt[:, k, idx * SB:(idx + 1) * SB],
                                wv_sb[:, k, :], start=(k == 0),
                                stop=(k == KCH - 1))
                        nc.vector.tensor_copy(
                            V_all[:, c * (TC // SB) + idx, :, 0:D],
                            v_ps[:].rearrange("p (h d) -> p h d",
                                              h=HEADS_PER_CORE))

                def a_units(c):
                    return ([lambda c=c: dma_x(c)] +
                            [lambda c=c, k=k, i=i: a_unit(c, k, i)
                             for k, i in [("q", 0), ("k", 0), ("v", 0),
                                          ("v", 1), ("q", 1), ("k", 1),
                                          ("v", 2), ("v", 3)]])

                # ---- phase C unit: out-proj for one t-block of chunk c ----
                def c_unit(c, tb4):
                    tb0 = c * TC + tb4 * SB
                    y_sb = ysbp.tile([128, 2, 512], F32, tag="ysb")
                    for e in range(2):
                        y_ps = ps_a.tile([128, 512], F32, tag="a")
                        for eh in range(2):
                            nc.tensor.matmul(
                                y_ps[:],
                                AT[eh][:, tb0:tb0 + SB],
                                wo_sb[:, eh, e * 512:(e + 1) * 512],
                                start=(eh == 0), stop=(eh == 1))
                        nc.vector.tensor_copy(y_sb[:, e, :], y_ps[:])
                    # y store on the SP HWDGE queue (SP engine is idle;
                    # keeps DMA triggers off the exp-busy ACT engine)
                    nc.sync.dma_start(
                        out=y[tb0:tb0 + SB, :], in_=y_sb[:].rearrange(
                            "p e f -> p (e f)"))

                def c_units(c):
                    return [lambda c=c, tb4=tb4: c_unit(c, tb4)
                            for tb4 in range(TC // SB)]

                # ---- phase B: attention blocks, software-pipelined ----
                # PV of block j is emitted only after S of block j+2 (a
                # global depth-2 pending queue spanning eh/chunk boundaries)
                # so the PE never head-of-line blocks on the ACT exp or on
                # the DVE draining the previous u2 accumulator.
                pend = []  # emit-closures for PV (+ r-chain) awaiting drain
                PEND_DEPTH = 4
                u2_box = [None]

                def emit_pv(c, eh, j, nblk, p2, off):
                    if j == 0:
                        u2_box[0] = ps_u.tile([D + 1, 2, TC], F32, tag="u",
                                              name=f"u{c}_{eh}")
                    u2 = u2_box[0]
                    for h2 in range(2):
                        nc.tensor.matmul(
                            u2[:, h2, off:],
                            V_all[:, j, 2 * eh + h2, :], p2[:, h2, off:],
                            start=(j == 0), stop=(j == nblk - 1))
                    if j == nblk - 1:
                        # softmax normalization for this (c, eh)
                        t0 = c * TC
                        r_row = rip.tile([1, 2, TC], F32, tag="rr")
                        nc.vector.tensor_copy(r_row[:], u2[D:D + 1, :, :])
                        rinv = rip.tile([1, 2, TC], F32, tag="ri")
                        nc.vector.reciprocal_approx_fast(
                            out=rinv[:], in_=r_row[:])
                        rb = rbp.tile([64, 2, TC], F32, tag="rb")
                        nc.gpsimd.partition_broadcast(rb[:], rinv[:])
                        for h2 in range(2):
                            r0 = 64 * h2
                            nc.vector.tensor_mul(
                                AT[eh][r0:r0 + 64, t0:t0 + TC],
                                u2[0:D, h2, :], rb[:, h2, :])

                def emit_b_chunk(c, fillers):
                    """Emit attention for chunk c, weaving `fillers` (list of
                    zero-arg closures of PE work) between blocks."""
                    t0 = c * TC
                    nblk = (c + 1) * (TC // SB)
                    blocks = [(eh, j) for eh in range(2) for j in range(nblk)]
                    nb = len(blocks)
                    nf = len(fillers)
                    fi = 0
                    # fillers may read data produced by PV/r-chain entries
                    # still pending from the previous chunk (e.g. out-proj
                    # reading AT): hold the weave until those have popped,
                    # else the reader is emitted before its writer and Tile
                    # can't see the dependency.
                    hold = len(pend)
                    nb_eff = nb - hold
                    for bi, (eh, j) in enumerate(blocks):
                        # scores for block (eh, j): valid cols t >= j*SB
                        off = max(0, j * SB - t0)
                        w = TC - off
                        s2 = ps_s.tile([128, 2, TC], F32, tag="s2")
                        for h2 in range(2):
                            r0 = 64 * h2
                            nc.tensor.matmul(
                                s2[:, h2, off:],
                                KT[eh][r0:r0 + 64, j * SB:(j + 1) * SB],
                                QT[eh][r0:r0 + 64, t0 + off:t0 + TC],
                                start=True, stop=True)
                        p2 = pp.tile([128, 2, TC], F32R, tag="pj")
                        nc.scalar.activation(
                            p2[:, :, off:], s2[:, :, off:],
                            mybir.ActivationFunctionType.Exp, scale=0.125)
                        if j >= c * (TC // SB):
                            # triangle: keep where (t0+off+y) >= (j*SB+x)
                            nc.gpsimd.affine_select(
                                out=p2[:, :, off:], in_=p2[:, :, off:],
                                compare_op=mybir.AluOpType.is_ge, fill=0.0,
                                base=t0 + off - j * SB,
                                pattern=[[0, 2], [1, w]],
                                channel_multiplier=-1)
                        pend.append(
                            lambda c=c, eh=eh, j=j, nblk=nblk, p2=p2, off=off:
                            emit_pv(c, eh, j, nblk, p2, off))
                        while len(pend) > PEND_DEPTH:
                            pend.pop(0)()
                        # weave fillers proportionally between blocks
                        while bi >= hold and fi * nb_eff < nf * (bi - hold + 1):
                            fillers[fi]()
                            fi += 1
                    while fi < nf:
                        fillers[fi]()
                        fi += 1

                # ---- main pipeline ----
                for u in a_units(0):
                    u()
                for c in range(NTC):
                    fillers = []
                    if c + 1 < NTC:
                        fillers += a_units(c + 1)
                    if c >= 1:
                        fillers += c_units(c - 1)
                    emit_b_chunk(c, fillers)
                while pend:
                    pend.pop(0)()
                for u in c_units(NTC - 1):
                    u()
                if debug_taps:
                    for eh in range(2):
                        nc.sync.dma_start(out=taps[f"qt{eh}"][:, :],
                                          in_=QT[eh][:].bitcast(F32))
                        nc.sync.dma_start(out=taps[f"kt{eh}"][:, :],
                                          in_=KT[eh][:].bitcast(F32))
                        nc.sync.dma_start(out=taps[f"at{eh}"][:, :],
                                          in_=AT[eh][:].bitcast(F32))
                    nc.sync.dma_start(out=taps["v"][:], in_=V_all[:].bitcast(F32))

            if repeat == 1:
                emit_body()
            else:
                with tc.For_i(0, repeat, 1):
                    emit_body()

    nc.compile()
    return nc


_NC_CACHE = {}


def _get_nc(repeat: int = 1):
    if repeat not in _NC_CACHE:
        _NC_CACHE[repeat] = build_kernel(repeat)
    return _NC_CACHE[repeat]


def make_in_maps(inputs: dict) -> list:
    x = np.asarray(inputs["x"], dtype=np.float32)
    Wq = np.asarray(inputs["Wq"], dtype=np.float32)
    Wk = np.asarray(inputs["Wk"], dtype=np.float32)
    Wv = np.asarray(inputs["Wv"], dtype=np.float32)
    Wo = np.asarray(inputs["Wo"], dtype=np.float32)
    bq = np.asarray(inputs["bq"], dtype=np.float32)

    in_maps = []
    for core in range(N_CORES):
        b, g = divmod(core, N_CORES // B)
        cs = slice(g * EPC, (g + 1) * EPC)
        in_maps.append({
            "xT": np.ascontiguousarray(x[b].T),
            "wq": np.ascontiguousarray(Wq[:, cs]),
            "wk": np.ascontiguousarray(Wk[:, cs]),
            "wv": np.ascontiguousarray(Wv[:, cs]),
            "wo": np.ascontiguousarray(Wo[cs, :]),
            "bq": np.ascontiguousarray(bq[cs]),
            "onesc": np.ones((128, NSB * HEADS_PER_CORE), dtype=np.float32),
        })
    return in_maps


def run_sharded(inputs: dict, trace: bool = False):
    """Shard inputs, run the SPMD kernel on 8 cores, unshard. Returns
    (output (B,T,E) float32, BassKernelResults)."""
    Wo = np.asarray(inputs["Wo"], dtype=np.float32)
    bv = np.asarray(inputs["bv"], dtype=np.float32)
    bo = np.asarray(inputs["bo"], dtype=np.float32)

    in_maps = make_in_maps(inputs)
    res = run_bass_kernel_spmd(_get_nc(), in_maps, core_ids=list(range(N_CORES)),
                               trace=trace)

    # unshard: sum the 4 head-group partials per batch; add the constant row
    # bo + bv @ Wo (v-bias commutes through the attention average exactly).
    const_row = (bo.astype(np.float64)
                 + bv.astype(np.float64) @ Wo.astype(np.float64))
    out = np.empty((B, T, E), dtype=np.float32)
    for b in range(B):
        acc = np.zeros((T, E), dtype=np.float64)
        for g in range(N_CORES // B):
            acc += res.results[b * (N_CORES // B) + g]["y"].astype(np.float64)
        out[b] = (acc + const_row).astype(np.float32)
    return out, res


def kernel(**inputs) -> np.ndarray:
    out, _ = run_sharded(inputs, trace=False)
    return out

